# revision 6
# baseline (speedup 1.0000x reference)
"""DirGCNConv Trainium2 Bass kernel (8 NeuronCores, SPMD).

Edge-list SpMM via gpsimd dma_gather + one-hot selector matmuls.
Gather descriptor generation is spread across all 4 SWDGE queues
(each queue runs on its own Q7 core pair), with larger gather calls
to amortize per-call fixed cost; ratio multiplies run on the Scalar
engine to unload the Vector engine.
"""
import sys

sys.path.insert(0, '/opt/trn_rl_repo')
import numpy as np

N = 100_000
F = 64
NCORES = 8
SHARD = N // NCORES              # 12500
WIN = 128                        # dests per window (selector M)
NWIN = (SHARD + WIN - 1) // WIN  # 98
NCHUNK = 4
CHUNK = N // NCHUNK              # 25000
CALL_TILES = 6                   # 768 gather indices per dma_gather call
ROWB = 128                       # table row elems: 2 streams x 64 f32 (512B)
NQUEUES = 4                      # SWDGE queues (Q7 core pairs)


def _inv_sqrt(d):
    return np.where(d > 0, 1.0 / np.sqrt(np.maximum(d, 1e-30)), 0.0).astype(np.float32)


def _host_schedules(edge_index):
    row = np.asarray(edge_index[0]).astype(np.int64)
    col = np.asarray(edge_index[1]).astype(np.int64)
    d_out = np.bincount(row, minlength=N).astype(np.float32)
    d_in = np.bincount(col, minlength=N).astype(np.float32)

    def Av(v):
        return np.bincount(row, weights=v[col], minlength=N).astype(np.float32)

    def Atv(v):
        return np.bincount(col, weights=v[row], minlength=N).astype(np.float32)

    iso, isi = _inv_sqrt(d_out), _inv_sqrt(d_in)
    scales = dict(
        iso=iso, isi=isi,
        sAAt=_inv_sqrt(Av(d_in)), sAtA=_inv_sqrt(Atv(d_out)),
        sAAo=_inv_sqrt(Av(d_out)), sAAi=_inv_sqrt(Atv(d_in)))
    ratio_row = (scales['sAAi'] / np.where(isi > 0, isi, 1.0)).astype(np.float32)
    ratio_col = (scales['sAAo'] / np.where(iso > 0, iso, 1.0)).astype(np.float32)

    def build_dir(dst, src, ratio):
        per_core = []
        cnt = np.zeros((NCORES, NWIN, NCHUNK), np.int64)
        for k in range(NCORES):
            lo = k * SHARD
            sel = (dst >= lo) & (dst < lo + SHARD)
            d = dst[sel] - lo
            s = src[sel]
            w = d // WIN
            c = s // CHUNK
            order = np.lexsort((s, w, c))
            per_core.append((d[order], s[order], w[order], c[order]))
            np.add.at(cnt[k], (w[order], c[order]), 1)
        tiles_wc = (cnt.max(0) + 127) // 128           # [NWIN, NCHUNK]
        tile_win, tile_chunk = [], []
        run_start = {}
        pos = 0
        for c in range(NCHUNK):
            for w in range(NWIN):
                run_start[(c, w)] = pos * 128
                tw = int(tiles_wc[w, c])
                tile_win += [w] * tw
                tile_chunk += [c] * tw
                pos += tw
        tile_win = np.array(tile_win, np.int64)
        tile_chunk = np.array(tile_chunk, np.int64)
        ntile = len(tile_win)
        idxs = np.zeros((NCORES, ntile * 128), np.int64)
        segids = np.full((NCORES, ntile * 128), -1.0, np.float32)
        ratios = np.zeros((NCORES, ntile * 128), np.float32)
        for k in range(NCORES):
            d, s, w, c = per_core[k]
            key = c * NWIN + w
            bnd = np.flatnonzero(np.diff(key)) + 1
            starts = np.concatenate([[0], bnd])
            ends = np.concatenate([bnd, [len(d)]])
            for a, b in zip(starts, ends):
                base = run_start[(c[a], w[a])]
                n = b - a
                idxs[k, base:base + n] = s[a:b] - c[a] * CHUNK
                segids[k, base:base + n] = (d[a:b] % WIN).astype(np.float32)
                ratios[k, base:base + n] = ratio[s[a:b]]
        return dict(tile_win=tile_win, tile_chunk=tile_chunk, ntile=ntile,
                    idxs=idxs, segids=segids, ratios=ratios)

    return scales, build_dir(row, col, ratio_row), build_dir(col, row, ratio_col)


def _call_plan(sched):
    """Static gather-call partition: list of (start_tile, ntiles, chunk)."""
    tc_, tw = sched['tile_chunk'], sched['ntile']
    plan = []
    t = 0
    while t < tw:
        c = tc_[t]
        n = 1
        while n < CALL_TILES and t + n < tw and tc_[t + n] == c:
            n += 1
        plan.append((t, n, int(c)))
        t += n
    return plan


def _wrap_idx_stream(idx_slots, plan):
    """Per-core [nslot] indices -> dma_gather SBUF layout [128, ntile*8] int16
    with per-call 16-partition wrapping, replicated to 128 partitions."""
    ntile = len(idx_slots) // 128
    out = np.zeros((128, ntile * 8), np.int16)
    for (t0, nt, _c) in plan:
        blk = idx_slots[t0 * 128:(t0 + nt) * 128]
        w = blk.reshape(nt * 8, 16).astype(np.int16).T      # [16, nt*8]
        out[:, t0 * 8:(t0 + nt) * 8] = np.tile(w, (8, 1))
    return out


# ---------------------------------------------------------------------------
_BUILT = None
TRACE = False          # test harness sets True (needs NTFF shim installed)
DEBUG = False
LAST_EXEC_NS = None
LAST_RESULTS = None


def _build(sr, sc):
    import concourse.bass as bass
    import concourse.bacc as bacc
    import concourse.mybir as mybir
    import concourse.tile as tile
    from concourse import library_config
    _f32 = mybir.dt.float32
    _i16 = mybir.dt.int16

    nt_r, nt_c = sr['ntile'], sc['ntile']
    plan_r, plan_c = _call_plan(sr), _call_plan(sc)

    nc = bacc.Bacc("TRN2", target_bir_lowering=False, debug=False,
                   num_devices=NCORES, num_swdge_queues=NQUEUES)
    D = {}
    D['t_row'] = nc.dram_tensor("t_row", [N, ROWB], _f32, kind="ExternalInput")
    D['t_col'] = nc.dram_tensor("t_col", [N, ROWB], _f32, kind="ExternalInput")
    D['idx_row'] = nc.dram_tensor("idx_row", [128, nt_r * 8], _i16, kind="ExternalInput")
    D['idx_col'] = nc.dram_tensor("idx_col", [128, nt_c * 8], _i16, kind="ExternalInput")
    D['seg_row'] = nc.dram_tensor("seg_row", [128, nt_r], _f32, kind="ExternalInput")
    D['seg_col'] = nc.dram_tensor("seg_col", [128, nt_c], _f32, kind="ExternalInput")
    D['rat_row'] = nc.dram_tensor("rat_row", [128, nt_r], _f32, kind="ExternalInput")
    D['rat_col'] = nc.dram_tensor("rat_col", [128, nt_c], _f32, kind="ExternalInput")
    D['oscale'] = nc.dram_tensor("oscale", [128, 6 * NWIN], _f32, kind="ExternalInput")
    D['Wcat'] = nc.dram_tensor("Wcat", [384, F], _f32, kind="ExternalInput")
    D['bias'] = nc.dram_tensor("bias", [128, F], _f32, kind="ExternalInput")
    D['iota'] = nc.dram_tensor("iota", [128, WIN], _f32, kind="ExternalInput")
    D['ident'] = nc.dram_tensor("ident", [128, 128], _f32, kind="ExternalInput")
    D['out'] = nc.dram_tensor("out", [SHARD, F], _f32, kind="ExternalOutput")
    u1d = nc.dram_tensor("u1d", [SHARD, F], _f32, kind="Internal")
    u2d = nc.dram_tensor("u2d", [SHARD, F], _f32, kind="Internal")
    ag_in = nc.dram_tensor("ag_in", [SHARD, 256], _f32, kind="Internal")
    ag_out = nc.dram_tensor("ag_out", [N, 256], _f32, kind="Internal",
                            addr_space="Shared")

    qctr = [0]

    with tile.TileContext(nc) as tc:
        import contextlib
        with contextlib.ExitStack() as ctx:
            gpool = ctx.enter_context(tc.tile_pool(name="g", bufs=4))
            ipool = ctx.enter_context(tc.tile_pool(name="ix", bufs=6))
            spool = ctx.enter_context(tc.tile_pool(name="sel", bufs=4))
            pspool = ctx.enter_context(tc.tile_pool(name="ps", bufs=2, space="PSUM"))
            ps2pool = ctx.enter_context(tc.tile_pool(name="ps2", bufs=2, space="PSUM"))
            cpool = ctx.enter_context(tc.tile_pool(name="const", bufs=1))
            apool = ctx.enter_context(tc.tile_pool(name="acc", bufs=1))
            hpool = ctx.enter_context(tc.tile_pool(name="h", bufs=2))

            nc.gpsimd.load_library(library_config.mlp)

            iota_sb = cpool.tile([128, WIN], _f32, tag="iota")
            nc.sync.dma_start(iota_sb[:], D['iota'][:, :])
            seg_sb = {}
            for nm, nt in (('seg_row', nt_r), ('seg_col', nt_c),
                           ('rat_row', nt_r), ('rat_col', nt_c)):
                t = cpool.tile([128, nt], _f32, tag=nm)
                nc.sync.dma_start(t[:], D[nm][:, :])
                seg_sb[nm] = t

            def gather_pass(sched, plan, table_fn, elem_step, idx_dram,
                            seg_t, rat_t, acc, outw, tag):
                tile_win, tile_chunk = sched['tile_win'], sched['tile_chunk']
                ntile = sched['ntile']
                cur_ps = [None, None]

                for (t0, ntc, c) in plan:
                    nidx = ntc * 128
                    q = qctr[0] % NQUEUES
                    qctr[0] += 1
                    ix = ipool.tile([128, CALL_TILES * 8], _i16, tag="ix")
                    nc.sync.dma_start(ix[:, 0:ntc * 8],
                                      idx_dram[:, t0 * 8: t0 * 8 + ntc * 8])
                    g = gpool.tile([128, CALL_TILES, ROWB], _f32, tag="g")
                    nc.gpsimd.dma_gather(
                        g[:, 0:ntc, :], table_fn(c), ix[:, 0:ntc * 8],
                        nidx, nidx, ROWB, elem_step=elem_step, queue_num=q)
                    for j in range(ntc):
                        tt = t0 + j
                        w = int(tile_win[tt])
                        first = (tt == 0) or (tile_win[tt - 1] != w) \
                            or (tile_chunk[tt - 1] != tile_chunk[tt])
                        last = (tt == ntile - 1) or (tile_win[tt + 1] != w) \
                            or (tile_chunk[tt + 1] != tile_chunk[tt])
                        s01 = spool.tile([128, WIN], _f32, tag="s01")
                        nc.vector.tensor_scalar(
                            s01[:], iota_sb[:], seg_t[:, tt:tt + 1], None,
                            mybir.AluOpType.is_equal)
                        if first:
                            cur_ps[0] = pspool.tile([128, ROWB], _f32,
                                                    name="psm", tag="psm")
                            if outw > ROWB:
                                cur_ps[1] = pspool.tile([128, F], _f32,
                                                        name="psr", tag="psr")
                        psm = cur_ps[0]
                        nc.tensor.matmul(psm[:], s01[:], g[:, j, :],
                                         start=first, stop=last)
                        if outw > ROWB:
                            g3 = gpool.tile([128, F], _f32, tag="g3")
                            nc.scalar.mul(g3[:], g[:, j, 0:F],
                                          rat_t[:, tt:tt + 1])
                            nc.tensor.matmul(cur_ps[1][:], s01[:], g3[:],
                                             start=first, stop=last)
                        if last:
                            nc.vector.tensor_add(
                                acc[:, w * outw:w * outw + ROWB],
                                acc[:, w * outw:w * outw + ROWB], psm[:])
                            if outw > ROWB:
                                nc.vector.tensor_add(
                                    acc[:, w * outw + ROWB:(w + 1) * outw],
                                    acc[:, w * outw + ROWB:(w + 1) * outw],
                                    cur_ps[1][:])

            # ---------------- phase 1 ----------------
            acc_a = apool.tile([128, NWIN * 192], _f32, tag="acc_a")
            nc.vector.memset(acc_a[:], 0.0)
            gather_pass(sr, plan_r,
                        lambda c: D['t_row'][c * CHUNK:(c + 1) * CHUNK, :],
                        ROWB, D['idx_row'],
                        seg_sb['seg_row'], seg_sb['rat_row'], acc_a, 192, "r1")
            # acc_a win cols: [u1|u4|u5] -> stash u4,u5 in ag_in; u1 -> u1d
            for w in range(NWIN):
                rows = min(128, SHARD - w * 128)
                b = w * 192
                nc.sync.dma_start(ag_in[w * 128:w * 128 + rows, 64:128],
                                  acc_a[0:rows, b + 128:b + 192])   # u5
                nc.sync.dma_start(ag_in[w * 128:w * 128 + rows, 128:192],
                                  acc_a[0:rows, b + 64:b + 128])    # u4
                nc.sync.dma_start(u1d[w * 128:w * 128 + rows, :],
                                  acc_a[0:rows, b:b + 64])          # u1
            acc_b = apool.tile([128, NWIN * 192], _f32, tag="acc_b")
            nc.vector.memset(acc_b[:], 0.0)
            gather_pass(sc, plan_c,
                        lambda c: D['t_col'][c * CHUNK:(c + 1) * CHUNK, :],
                        ROWB, D['idx_col'],
                        seg_sb['seg_col'], seg_sb['rat_col'], acc_b, 192, "c1")
            # acc_b win cols: [u2|u3|u6]
            for w in range(NWIN):
                rows = min(128, SHARD - w * 128)
                b = w * 192
                nc.sync.dma_start(ag_in[w * 128:w * 128 + rows, 0:64],
                                  acc_b[0:rows, b + 64:b + 128])    # u3
                nc.sync.dma_start(ag_in[w * 128:w * 128 + rows, 192:256],
                                  acc_b[0:rows, b + 128:b + 192])   # u6
                nc.sync.dma_start(u2d[w * 128:w * 128 + rows, :],
                                  acc_b[0:rows, b:b + 64])          # u2
            # ---------------- AllGather ----------------
            nc.gpsimd.collective_compute(
                "AllGather", mybir.AluOpType.bypass,
                ins=[ag_in[:, :].opt()],
                outs=[ag_out[:, :].opt()],
                replica_groups=[list(range(NCORES))],
            )

            # ---------------- phase 2 ----------------
            # reuse acc_a / acc_b buffers (first 128*NWIN cols)
            nc.vector.memset(acc_a[:], 0.0)
            nc.vector.memset(acc_b[:], 0.0)
            gather_pass(sr, plan_r,
                        lambda c: ag_out[c * CHUNK:(c + 1) * CHUNK, 0:128],
                        256, D['idx_row'],
                        seg_sb['seg_row'], None, acc_a, 128, "r2")
            gather_pass(sc, plan_c,
                        lambda c: ag_out[c * CHUNK:(c + 1) * CHUNK, 128:256],
                        256, D['idx_col'],
                        seg_sb['seg_col'], None, acc_b, 128, "c2")
            # acc_a win cols: [AAt|AA] ; acc_b win cols: [AtA|AtAt]

            # ---------------- final combine ----------------
            wcat_sb = cpool.tile([128, 3 * F], _f32, tag="wcat")
            for k in range(3):
                nc.sync.dma_start(wcat_sb[:, k * F:(k + 1) * F],
                                  D['Wcat'][k * 128:(k + 1) * 128, :])
            bias_sb = cpool.tile([128, F], _f32, tag="bias")
            nc.sync.dma_start(bias_sb[:], D['bias'][:, :])
            ident_sb = cpool.tile([128, 128], _f32, tag="ident")
            nc.sync.dma_start(ident_sb[:], D['ident'][:, :])
            osc_sb = cpool.tile([128, 6 * NWIN], _f32, tag="osc")
            nc.sync.dma_start(osc_sb[:], D['oscale'][:, :])

            for w in range(NWIN):
                rows = min(128, SHARD - w * 128)
                h = hpool.tile([128, 6 * F], _f32, tag="h")
                u1t = hpool.tile([128, F], _f32, tag="u1t")
                nc.sync.dma_start(u1t[0:rows, :], u1d[w * 128:w * 128 + rows, :])
                u2t = hpool.tile([128, F], _f32, tag="u2t")
                nc.sync.dma_start(u2t[0:rows, :], u2d[w * 128:w * 128 + rows, :])
                # H blocks in Wcat row order: A_x, At_x, AAt, AtA, AA, AtAt
                srcs = [
                    (u1t, 0, 0),                 # A_x  = iso  * u1
                    (u2t, 0, 1),                 # At_x = isi  * u2
                    (acc_a, w * 128 + 0, 2),     # AAt  = sAAt * .
                    (acc_b, w * 128 + 0, 3),     # AtA  = sAtA * .
                    (acc_a, w * 128 + 64, 4),    # AA   = sAAo * .
                    (acc_b, w * 128 + 64, 5),    # AtAt = sAAi * .
                ]
                for i, (src, off, sidx) in enumerate(srcs):
                    nc.scalar.mul(
                        h[:, i * F:(i + 1) * F], src[:, off:off + F],
                        osc_sb[:, sidx * NWIN + w:sidx * NWIN + w + 1])
                ps_out = ps2pool.tile([128, F], _f32, tag="ps_out")
                for k in range(3):
                    hT_ps = ps2pool.tile([128, 128], _f32, tag="hT_ps")
                    nc.tensor.transpose(hT_ps[:], h[:, k * 128:(k + 1) * 128],
                                        ident_sb[:])
                    hT = hpool.tile([128, 128], _f32, tag="hT")
                    nc.vector.tensor_copy(hT[:], hT_ps[:])
                    nc.tensor.matmul(ps_out[:], hT[:],
                                     wcat_sb[:, k * F:(k + 1) * F],
                                     start=(k == 0), stop=(k == 2))
                o = hpool.tile([128, F], _f32, tag="o")
                nc.vector.tensor_add(o[:], ps_out[:], bias_sb[:])
                nc.sync.dma_start(D['out'][w * 128:w * 128 + rows, :],
                                  o[0:rows, :])

    nc.compile()
    return nc


def kernel(x, edge_index, W_sd, b_sd, W_ds, b_ds, W0, b0, W1, b1, W2, b2,
           W3, b3):
    global _BUILT
    from concourse import bass_utils

    x = np.asarray(x, dtype=np.float32)
    scales, sr, sc = _host_schedules(edge_index)
    if _BUILT is None:
        _BUILT = _build(sr, sc)
    nc = _BUILT

    iso, isi = scales['iso'], scales['isi']
    t_row = np.concatenate([isi[:, None] * x, scales['sAtA'][:, None] * x],
                           1).astype(np.float32)
    t_col = np.concatenate([iso[:, None] * x, scales['sAAt'][:, None] * x],
                           1).astype(np.float32)
    Wcat = 0.75 * np.concatenate(
        [W_sd, W_ds, W0, W1, W2, W3], 0).astype(np.float32)
    bias = np.tile((0.75 * (np.asarray(b_sd) + np.asarray(b_ds) + np.asarray(b0)
                            + np.asarray(b1) + np.asarray(b2)
                            + np.asarray(b3))).astype(np.float32)[None, :],
                   (128, 1))
    iota = np.tile(np.arange(WIN, dtype=np.float32)[None, :], (128, 1))
    ident = np.eye(128, dtype=np.float32)

    plan_r, plan_c = _call_plan(sr), _call_plan(sc)
    # outer scale layout: [128, 6*NWIN], node w*128+p -> col sidx*NWIN+w
    order = ('iso', 'isi', 'sAAt', 'sAtA', 'sAAo', 'sAAi')
    in_maps = []
    for k in range(NCORES):
        sl = slice(k * SHARD, (k + 1) * SHARD)
        osc = np.zeros((128, 6 * NWIN), np.float32)
        for sidx, nm in enumerate(order):
            v = np.zeros(NWIN * 128, np.float32)
            v[:SHARD] = scales[nm][sl]
            osc[:, sidx * NWIN:(sidx + 1) * NWIN] = v.reshape(NWIN, 128).T
        in_maps.append({
            't_row': t_row, 't_col': t_col,
            'idx_row': _wrap_idx_stream(sr['idxs'][k], plan_r),
            'idx_col': _wrap_idx_stream(sc['idxs'][k], plan_c),
            'seg_row': sr['segids'][k].reshape(-1, 128).T.copy(),
            'seg_col': sc['segids'][k].reshape(-1, 128).T.copy(),
            'rat_row': sr['ratios'][k].reshape(-1, 128).T.copy(),
            'rat_col': sc['ratios'][k].reshape(-1, 128).T.copy(),
            'oscale': osc, 'Wcat': Wcat, 'bias': bias,
            'iota': iota, 'ident': ident,
        })
    res = bass_utils.run_bass_kernel_spmd(
        nc, in_maps, core_ids=list(range(NCORES)), trace=TRACE)
    global LAST_EXEC_NS, LAST_RESULTS
    LAST_EXEC_NS = res.exec_time_ns
    LAST_RESULTS = res.results
    out = np.concatenate([r['out'] for r in res.results], 0)
    return out
